# revision 20
# baseline (speedup 1.0000x reference)
"""AdditiveAttention TRN2 kernel (8 NeuronCores, data-parallel over batch).

Reference computation per batch b:
    q = queries[b] @ W_q                       # [Lq, H]
    k = key[b] @ W_k                           # [Lk, H]
    scores[i, j] = sum_h W_v[h] * tanh(q[i,h] + k[j,h])
    scores[:, j >= valid_length[b]] = -1e6
    out[b] = softmax(scores, axis=-1) @ value[b]

Device strategy (per core = one batch):
  - Layout: H=128 on SBUF partitions. qT [H, Lq], kT [H, Lk] computed on
    device from host-pre-transposed queries/key (packed with W_q/W_k so
    each projection matmul depends on a single DMA).
  - Per query i: ScalarE ACTIVATE tanh(kT + bias=qT[:, i]) -> F_i [H, Lk]
    in bf16.
  - Scores: TensorE matmul with a sliding "hot column" window of a
    [H, 256] buffer that is all zeros except column 128 = W_v. The window
    whot[:, 128-i : 256-i] has W_v in column i, so the matmul routes query
    i's scores to PSUM partition i; 128 accumulating matmuls build
    scores [128 queries, Lk] with zero cross-talk.
  - Softmax without max-subtraction: |scores| <= sum|W_v| ~ 9, exp is safe
    in fp32. Masking/denominator: value matrix gets a 257th column = 1 on
    valid rows, 0 on invalid; invalid value rows are zeroed host-side. Then
    attn_unnorm @ [value | ind] yields numerator and denominator in one
    matmul; invalid keys contribute to neither.
  - attn^T for the AV matmul comes from DMA transpose (bf16 XBAR).

Hardware constraint honored throughout: a PE matmul can carry at most ONE
sync wait, so every matmul's dependencies are arranged to arrive through
a single semaphore (packed DMAs; projection PSUM->SBUF copies on ScalarE
so WAR and RAW deps collapse into the Activation semaphore).
"""

import ml_dtypes
import numpy as np

from concourse import bacc, bass, mybir
from concourse import tile
from concourse.bass_utils import run_bass_kernel_spmd

B, LQ, LK, QS, KS, H, VS = 8, 256, 1024, 256, 256, 128, 256
F32 = mybir.dt.float32
BF16 = mybir.dt.bfloat16

_CACHE: dict = {}


def _build():
    nc = bacc.Bacc("TRN2", target_bir_lowering=False, debug=False)
    # qpack rows d in [0, 256): [W_q[d, :] | queries[:, d]] -> [256, H + LQ]
    qpack = nc.declare_dram_parameter("qpack", [QS, H + LQ], F32, isOutput=False)
    # kpack rows d: [W_k[d, :] | key[:, d]] -> [256, H + LK]
    kpack = nc.declare_dram_parameter("kpack", [KS, H + LK], F32, isOutput=False)
    vals = nc.declare_dram_parameter("vals", [LK, VS + 1], BF16, isOutput=False)
    whot = nc.declare_dram_parameter("whot", [H, 256], BF16, isOutput=False)
    out = nc.declare_dram_parameter("out", [LQ, VS], F32, isOutput=True)

    NKC = LK // 128  # key chunks of 128

    with tile.TileContext(nc) as tc:
        with (
            tc.tile_pool(name="const", bufs=1) as cpool,
            tc.tile_pool(name="fbuf", bufs=3) as fpool,
            tc.tile_pool(name="exps", bufs=2) as epool,
            tc.tile_pool(name="expt", bufs=2) as etpool,
            tc.tile_pool(name="outs", bufs=2) as opool,
            tc.tile_pool(name="scal", bufs=2) as spool,
            tc.tile_pool(name="ps_sc", bufs=4, space="PSUM") as ps_sc,
            tc.tile_pool(name="ps_av", bufs=2, space="PSUM") as ps_av,
        ):
            # ---- DMA inputs to SBUF ----
            qpack_sb = cpool.tile([128, 2, H + LQ], F32)
            kpack_sb = cpool.tile([128, 2, H + LK], F32)
            vals_sb = cpool.tile([128, NKC, VS + 1], BF16)
            whot_sb = cpool.tile([128, 256], BF16)
            for d in range(2):
                nc.sync.dma_start(out=qpack_sb[:, d, :],
                                  in_=qpack[128 * d:128 * (d + 1), :])
                nc.sync.dma_start(out=kpack_sb[:, d, :],
                                  in_=kpack[128 * d:128 * (d + 1), :])
            for c in range(NKC):
                nc.sync.dma_start(out=vals_sb[:, c, :],
                                  in_=vals[128 * c:128 * (c + 1), :])
            nc.sync.dma_start(out=whot_sb[:], in_=whot[:])

            # ---- projections: qT [H, Lq], kT [H, Lk] ----
            # PSUM->SBUF copies go through ScalarE so downstream matmuls'
            # WAR deps collapse into the Activation semaphore.
            qT = cpool.tile([128, LQ], F32)
            kT = cpool.tile([128, LK], F32)
            qT_ps = ps_sc.tile([128, 512], F32, tag="ps_sc")
            for d in range(2):
                nc.tensor.matmul(qT_ps[:, 0:LQ],
                                 qpack_sb[:, d, 0:H],
                                 qpack_sb[:, d, H:H + LQ],
                                 start=(d == 0), stop=(d == 1))
            nc.scalar.copy(qT[:], qT_ps[:, 0:LQ])
            for half in range(2):
                kT_ps = ps_sc.tile([128, 512], F32, tag="ps_sc")
                for d in range(2):
                    nc.tensor.matmul(
                        kT_ps[:],
                        kpack_sb[:, d, 0:H],
                        kpack_sb[:, d, H + 512 * half:H + 512 * (half + 1)],
                        start=(d == 0), stop=(d == 1))
                nc.scalar.copy(kT[:, 512 * half:512 * (half + 1)], kT_ps[:])

            # ---- main loop over query blocks ----
            for qb in range(LQ // 128):
                sc_a = ps_sc.tile([128, 512], F32, tag="ps_sc")
                sc_b = ps_sc.tile([128, 512], F32, tag="ps_sc")
                for i in range(128):
                    qidx = qb * 128 + i
                    F = fpool.tile([128, LK], BF16, tag="fbuf")
                    nc.scalar.activation(
                        F[:], kT[:],
                        mybir.ActivationFunctionType.Tanh,
                        bias=qT[:, qidx:qidx + 1])
                    wr = whot_sb[:, 128 - i:256 - i]
                    nc.tensor.matmul(sc_a[:], wr, F[:, 0:512],
                                     start=(i == 0), stop=(i == 127))
                    nc.tensor.matmul(sc_b[:], wr, F[:, 512:1024],
                                     start=(i == 0), stop=(i == 127))

                expS = epool.tile([128, LK], BF16, tag="exps")
                nc.scalar.activation(expS[:, 0:512], sc_a[:],
                                     mybir.ActivationFunctionType.Exp)
                nc.scalar.activation(expS[:, 512:1024], sc_b[:],
                                     mybir.ActivationFunctionType.Exp)

                expT = etpool.tile([128, NKC, 128], BF16, tag="expt")
                for c in range(NKC):
                    nc.sync.dma_start_transpose(
                        expT[:, c, :], expS[:, 128 * c:128 * (c + 1)])

                av = ps_av.tile([128, VS + 1], F32, tag="ps_av")
                for c in range(NKC):
                    nc.tensor.matmul(av[:], expT[:, c, :], vals_sb[:, c, :],
                                     start=(c == 0), stop=(c == NKC - 1))

                r = spool.tile([128, 1], F32, tag="scal")
                nc.vector.reciprocal(r[:], av[:, VS:VS + 1])
                o_sb = opool.tile([128, VS], F32, tag="outs")
                nc.vector.tensor_scalar_mul(o_sb[:], av[:, 0:VS], r[:])
                nc.sync.dma_start(out=out[qb * 128:(qb + 1) * 128, :], in_=o_sb[:])

    nc.compile()
    return nc


def _make_in_maps(inputs) -> list[dict]:
    queries = np.ascontiguousarray(np.asarray(inputs["queries"], dtype=np.float32))
    key = np.ascontiguousarray(np.asarray(inputs["key"], dtype=np.float32))
    value = np.ascontiguousarray(np.asarray(inputs["value"], dtype=np.float32))
    vl = np.asarray(inputs["valid_length"], dtype=np.int32)
    W_q = np.ascontiguousarray(np.asarray(inputs["W_q"], dtype=np.float32))
    W_k = np.ascontiguousarray(np.asarray(inputs["W_k"], dtype=np.float32))
    W_v = np.asarray(inputs["W_v"], dtype=np.float32)

    whot = np.zeros((H, 256), dtype=np.float32)
    whot[:, 128] = W_v
    whot = whot.astype(ml_dtypes.bfloat16)

    in_maps = []
    for b in range(B):
        v = max(int(vl[b]), 0)
        vals = np.zeros((LK, VS + 1), dtype=np.float32)
        vals[:v, :VS] = value[b, :v]
        vals[:v, VS] = 1.0
        vals = vals.astype(ml_dtypes.bfloat16)
        qpack = np.concatenate([W_q, queries[b].T], axis=1)
        kpack = np.concatenate([W_k, key[b].T], axis=1)
        in_maps.append({
            "qpack": np.ascontiguousarray(qpack),
            "kpack": np.ascontiguousarray(kpack),
            "vals": vals,
            "whot": whot,
        })
    return in_maps


def _postprocess(res, inputs) -> np.ndarray:
    value = np.asarray(inputs["value"], dtype=np.float32)
    vl = np.asarray(inputs["valid_length"], dtype=np.int32)
    out = np.stack([np.asarray(res.results[i]["out"]) for i in range(B)], axis=0)
    # valid_length == 0: reference's masked softmax degenerates to uniform
    # attention over all Lk keys (all scores equal) -> mean of values.
    for b in range(B):
        if int(vl[b]) <= 0:
            out[b] = value[b].mean(axis=0, keepdims=True)
    return out.astype(np.float32)


def kernel(**inputs) -> np.ndarray:
    if "nc" not in _CACHE:
        _CACHE["nc"] = _build()
    nc = _CACHE["nc"]
    in_maps = _make_in_maps(inputs)
    res = run_bass_kernel_spmd(nc, in_maps, core_ids=list(range(B)))
    return _postprocess(res, inputs)


# revision 21
# speedup vs baseline: 1.0638x; 1.0638x over previous
"""AdditiveAttention TRN2 kernel v4 — sin-basis scores with binade-mask
range reduction.

tanh(s) ~= sum_m c_m sin(w_m s) on [-10.5, 10.5] (M=11, sup 1.9e-3), so

  scores[i,j] = sum_h W_v[h] tanh(q_ih + k_jh)
             ~= sum_m sum_h [c_m W_v[h] sin(w_m q)] cos(w_m k)
                          + [c_m W_v[h] cos(w_m q)] sin(w_m k)

i.e. 2M rank-128 matmuls instead of a 33.5M-element tanh.

HW Sin is only valid on [-pi, pi], so phases are range-reduced exactly
with an fp32 binade trick: PE computes p = (w_m/2pi) * x via pre-scaled
f32r projection weights; DVE adds 24.0 (sin) / 24.25 (cos) putting
s1 = p + 24 in the [16, 32) binade, where frac(s1) occupies the low 19
mantissa bits; GPSIMD extracts them (bitwise_and 0x7FFFF); ScalarE
evaluates Sin(m * 2pi/2^19 - pi) = -sin(w x [+ pi/2]).  Both sides of
every product carry the minus sign, so the signs cancel.

Softmax/masking identical to v1: no max-subtraction (|scores| <= ~9),
indicator column in the value matrix gives the denominator, invalid
value rows zeroed host-side; attn^T via DMA transpose (bf16 XBAR).
"""

import math

import ml_dtypes
import numpy as np

from concourse import bacc, mybir
from concourse import tile
from concourse.bass_utils import run_bass_kernel_spmd

B, LQ, LK, QS, KS, H, VS = 8, 256, 1024, 256, 256, 128, 256
F32 = mybir.dt.float32
F32R = mybir.dt.float32r
I32 = mybir.dt.int32
BF16 = mybir.dt.bfloat16

W_FIT = [0.0822537725, -0.298217301, -0.142006636, 0.7778114887,
         1.2988701126, 1.8225811398, 1.1451769858, 2.3609781773,
         3.4643752598, 2.9094341665, 4.0118596954]
C_FIT = [0.2297757049, -0.8389809546, -0.4004822335, 0.3248009122,
         0.1335364513, 0.0610199843, 0.0074227805, 0.026674116,
         0.0048070768, 0.0114453089, 0.0018884206]
M = len(W_FIT)

SCALE_SIN = 2.0 * math.pi / (1 << 19)

_CACHE: dict = {}


def _build():
    nc = bacc.Bacc("TRN2", target_bir_lowering=False, debug=False)
    qTd = nc.declare_dram_parameter("qTd", [QS, LQ], F32R, isOutput=False)
    kTd = nc.declare_dram_parameter("kTd", [KS, LK], F32R, isOutput=False)
    # wqm[d, m, h] = (w_m / 2pi) * W_q[d, h]
    wqm = nc.declare_dram_parameter("wqm", [QS, M, H], F32R, isOutput=False)
    wkm = nc.declare_dram_parameter("wkm", [KS, M, H], F32R, isOutput=False)
    cw = nc.declare_dram_parameter("cw", [H, M], F32, isOutput=False)
    negpi = nc.declare_dram_parameter("negpi", [H, 1], F32, isOutput=False)
    vals = nc.declare_dram_parameter("vals", [LK, VS + 1], BF16, isOutput=False)
    out = nc.declare_dram_parameter("out", [LQ, VS], F32, isOutput=True)

    NKC = LK // 128
    SIN = mybir.ActivationFunctionType.Sin
    EXP = mybir.ActivationFunctionType.Exp
    AND = mybir.AluOpType.bitwise_and
    ADD = mybir.AluOpType.add

    with tile.TileContext(nc) as tc:
        with (
            tc.tile_pool(name="const", bufs=1) as cpool,
            tc.tile_pool(name="aff", bufs=2) as apool,
            tc.tile_pool(name="msk", bufs=2) as mpool,
            tc.tile_pool(name="basis", bufs=2) as bpool,
            tc.tile_pool(name="exps", bufs=2) as epool,
            tc.tile_pool(name="expt", bufs=2) as etpool,
            tc.tile_pool(name="outs", bufs=2) as opool,
            tc.tile_pool(name="scal", bufs=2) as spool,
            tc.tile_pool(name="ps_k", bufs=1, space="PSUM") as ps_k,
            tc.tile_pool(name="ps_q", bufs=2, space="PSUM") as ps_q,
            tc.tile_pool(name="ps_sc", bufs=4, space="PSUM") as ps_sc,
        ):
            kTd_sb = cpool.tile([128, 2, LK], F32R)
            qTd_sb = cpool.tile([128, 2, LQ], F32R)
            wkm_sb = cpool.tile([128, 2, M, H], F32R)
            wqm_sb = cpool.tile([128, 2, M, H], F32R)
            cw_sb = cpool.tile([128, M], F32)
            negpi_sb = cpool.tile([128, 1], F32)
            vals_sb = cpool.tile([128, NKC, VS + 1], BF16)
            for d in range(2):
                nc.sync.dma_start(out=kTd_sb[:, d, :], in_=kTd[128 * d:128 * (d + 1), :])
                nc.sync.dma_start(out=qTd_sb[:, d, :], in_=qTd[128 * d:128 * (d + 1), :])
                nc.sync.dma_start(out=wkm_sb[:, d], in_=wkm[128 * d:128 * (d + 1)])
                nc.sync.dma_start(out=wqm_sb[:, d], in_=wqm[128 * d:128 * (d + 1)])
            nc.sync.dma_start(out=cw_sb[:], in_=cw[:])
            nc.sync.dma_start(out=negpi_sb[:], in_=negpi[:])
            for c in range(NKC):
                nc.sync.dma_start(out=vals_sb[:, c, :], in_=vals[128 * c:128 * (c + 1), :])

            sc = [[ps_sc.tile([128, 512], F32, tag="ps_sc", name=f"sc{qb}{hf}")
                   for hf in range(2)] for qb in range(2)]

            for m in range(M):
                # p = (w_m/2pi) * x   (PE, f32r)
                kps = ps_k.tile([128, 2, 512], F32, tag="ps_k")
                for half in range(2):
                    for d in range(2):
                        nc.tensor.matmul(
                            kps[:, half], wkm_sb[:, d, m, :],
                            kTd_sb[:, d, 512 * half:512 * (half + 1)],
                            start=(d == 0), stop=(d == 1))
                qps = ps_q.tile([128, 256], F32, tag="ps_q")
                for d in range(2):
                    nc.tensor.matmul(qps[:], wqm_sb[:, d, m, :], qTd_sb[:, d, :],
                                     start=(d == 0), stop=(d == 1))

                # s1 = p + 24 (sin) / 24.25 (cos) -> [16, 32) binade (DVE);
                # sin half in [:, 0, :], cos half in [:, 1, :]
                s_k = apool.tile([128, 2, LK], F32, tag="s_k")
                s_q = apool.tile([128, 2, LQ], F32, tag="s_q")
                nc.vector.tensor_scalar(s_k[:, 0, :], kps[:, :, :], 24.0, None, ADD)
                nc.vector.tensor_scalar(s_k[:, 1, :], kps[:, :, :], 24.25, None, ADD)
                nc.vector.tensor_scalar(s_q[:, 0, :], qps[:], 24.0, None, ADD)
                nc.vector.tensor_scalar(s_q[:, 1, :], qps[:], 24.25, None, ADD)

                # frac bits (DVE): m = bits(s1) & 0x7FFFF -- one op per side
                m_k = mpool.tile([128, 2, LK], I32, tag="m_k")
                m_q = mpool.tile([128, 2, LQ], I32, tag="m_q")
                nc.vector.tensor_scalar(m_k[:], s_k[:].bitcast(I32), 0x7FFFF, None, AND)
                nc.vector.tensor_scalar(m_q[:], s_q[:].bitcast(I32), 0x7FFFF, None, AND)

                # basis = Sin(m * 2pi/2^19 - pi) = -sin(w x [+pi/2])  (ScalarE)
                bas_k = bpool.tile([128, 2, LK], BF16, tag="bas_k")
                bas_q = bpool.tile([128, 2, LQ], BF16, tag="bas_q")
                nc.scalar.activation(bas_k[:], m_k[:], SIN, scale=SCALE_SIN, bias=negpi_sb[:])
                nc.scalar.activation(bas_q[:], m_q[:], SIN, scale=SCALE_SIN, bias=negpi_sb[:])
                ks, kc = bas_k[:, 0, :], bas_k[:, 1, :]
                qs, qc = bas_q[:, 0, :], bas_q[:, 1, :]

                # weight q-side (DVE): qsw = qs * (c_m W_v)
                qsw = bpool.tile([128, LQ], BF16, tag="qsw")
                qcw = bpool.tile([128, LQ], BF16, tag="qcw")
                nc.vector.tensor_scalar_mul(qsw[:], qs, cw_sb[:, m:m + 1])
                nc.vector.tensor_scalar_mul(qcw[:], qc, cw_sb[:, m:m + 1])

                # scores += qsw^T kc + qcw^T ks  (PE, bf16; signs cancel)
                for qb in range(2):
                    for half in range(2):
                        nc.tensor.matmul(
                            sc[qb][half][:], qsw[:, 128 * qb:128 * (qb + 1)],
                            bas_k[:, 1, 512 * half:512 * (half + 1)],
                            start=(m == 0), stop=False)
                        nc.tensor.matmul(
                            sc[qb][half][:], qcw[:, 128 * qb:128 * (qb + 1)],
                            bas_k[:, 0, 512 * half:512 * (half + 1)],
                            start=False, stop=(m == M - 1))

            for qb in range(2):
                expS = epool.tile([128, LK], BF16, tag="exps")
                for half in range(2):
                    nc.scalar.activation(expS[:, 512 * half:512 * (half + 1)],
                                         sc[qb][half][:], EXP)
                expT = etpool.tile([128, NKC, 128], BF16, tag="expt")
                for c in range(NKC):
                    nc.sync.dma_start_transpose(
                        expT[:, c, :], expS[:, 128 * c:128 * (c + 1)])
                av = ps_q.tile([128, VS + 1], F32, tag="ps_q")
                for c in range(NKC):
                    nc.tensor.matmul(av[:], expT[:, c, :], vals_sb[:, c, :],
                                     start=(c == 0), stop=(c == NKC - 1))
                r = spool.tile([128, 1], F32, tag="scal")
                nc.vector.reciprocal(r[:], av[:, VS:VS + 1])
                o_sb = opool.tile([128, VS], F32, tag="outs")
                nc.vector.tensor_scalar_mul(o_sb[:], av[:, 0:VS], r[:])
                nc.sync.dma_start(out=out[qb * 128:(qb + 1) * 128, :], in_=o_sb[:])

    nc.compile()
    return nc


def _make_in_maps(inputs) -> list[dict]:
    queries = np.ascontiguousarray(np.asarray(inputs["queries"], dtype=np.float32))
    key = np.ascontiguousarray(np.asarray(inputs["key"], dtype=np.float32))
    value = np.ascontiguousarray(np.asarray(inputs["value"], dtype=np.float32))
    vl = np.asarray(inputs["valid_length"], dtype=np.int32)
    W_q = np.asarray(inputs["W_q"], dtype=np.float32)
    W_k = np.asarray(inputs["W_k"], dtype=np.float32)
    W_v = np.asarray(inputs["W_v"], dtype=np.float32)

    wfit = np.asarray(W_FIT, np.float32)
    cfit = np.asarray(C_FIT, np.float32)
    s = wfit / (2.0 * math.pi)
    wqm = np.ascontiguousarray((W_q[:, None, :] * s[None, :, None]).astype(np.float32))
    wkm = np.ascontiguousarray((W_k[:, None, :] * s[None, :, None]).astype(np.float32))
    cw = np.ascontiguousarray((W_v[:, None] * cfit[None, :]).astype(np.float32))
    negpi = np.full((H, 1), -math.pi, np.float32)

    in_maps = []
    for b in range(B):
        v = max(int(vl[b]), 0)
        vals = np.zeros((LK, VS + 1), dtype=np.float32)
        vals[:v, :VS] = value[b, :v]
        vals[:v, VS] = 1.0
        vals = vals.astype(ml_dtypes.bfloat16)
        in_maps.append({
            "qTd": np.ascontiguousarray(queries[b].T),
            "kTd": np.ascontiguousarray(key[b].T),
            "wqm": wqm, "wkm": wkm, "cw": cw, "negpi": negpi,
            "vals": vals,
        })
    return in_maps


def _postprocess(res, inputs) -> np.ndarray:
    value = np.asarray(inputs["value"], dtype=np.float32)
    vl = np.asarray(inputs["valid_length"], dtype=np.int32)
    out = np.stack([np.asarray(res.results[i]["out"]) for i in range(B)], axis=0)
    for b in range(B):
        if int(vl[b]) <= 0:
            out[b] = value[b].mean(axis=0, keepdims=True)
    return out.astype(np.float32)


def kernel(**inputs) -> np.ndarray:
    if "nc" not in _CACHE:
        _CACHE["nc"] = _build()
    nc = _CACHE["nc"]
    in_maps = _make_in_maps(inputs)
    res = run_bass_kernel_spmd(nc, in_maps, core_ids=list(range(B)))
    return _postprocess(res, inputs)
